# revision 1
# baseline (speedup 1.0000x reference)
"""ContraNorm (NormLayer 'CN' branch) on 8 Trainium2 NeuronCores.

kernel(x, adj) -> (1+s)*x - s * softmax(mask(cossim(x, x))) @ x  with s=1.

Strategy: shard queries (rows) across the 8 cores (1280 rows each, N padded
10000 -> 10240); every core holds all keys. Each core runs a flash-attention
style loop over key tiles computing simT = (x_keys @ qn^T) in [key, query]
layout, exp with the per-key 1/|x_k| folded in as a per-partition activation
scale, a (1-adj)^T bf16 mask multiply (exact zeros, matching -inf softmax),
then accumulates p^T @ x and the softmax denominator (p^T @ ones) in PSUM over
all key tiles.  Cosines lie in [-1, 1] so exp never overflows and no
max-subtraction pass is needed.  Matmuls run in bf16 (full PE rate) with fp32
PSUM accumulation; norms / exp / final combine run in fp32.
"""
import os
import sys

sys.path.insert(0, '/opt/trn_rl_repo')

from contextlib import ExitStack

import numpy as np
import ml_dtypes

import concourse.bass as bass
import concourse.tile as tile
from concourse import mybir
from concourse.masks import make_identity

F32 = mybir.dt.float32
BF16 = mybir.dt.bfloat16
U8 = mybir.dt.uint8
AF = mybir.ActivationFunctionType
ALU = mybir.AluOpType

N = 10000
D = 512
N_CORES = 8
N_PAD = 10240            # 80 key tiles of 128; == 8 * 1280
NQ = N_PAD // N_CORES    # 1280 query rows per core (core 7: 1040 real)


def _split_excess_waits(nc, max_waits=1):
    """Walrus CoreV3 rejects >1 sync wait per CTRL instruction; Tile's tail
    drain carries one wait per outstanding engine/DMA queue.  Hoist monotone
    (sem-ge) waits onto same-engine NoOps placed immediately before the
    offending instruction — semantically identical, since the engine executes
    them in program order."""
    for f in nc.m.functions:
        for bb in f.blocks:
            insts = list(bb.instructions)
            new_insts = []
            changed = False
            for inst in insts:
                si = inst.sync_info
                waits = list(si.on_wait) if si is not None else []
                if len(waits) > max_waits:
                    ge = [w for w in waits if "eq" not in (w.wait_mode or "")]
                    eq = [w for w in waits if "eq" in (w.wait_mode or "")]
                    keep_n = max(max_waits - len(eq), 0)
                    n_extra = max(len(ge) - keep_n, 0)
                    extra, keep = ge[:n_extra], ge[n_extra:] + eq
                    if len(keep) > max_waits:
                        raise RuntimeError(
                            f"{inst.name}: non-monotone waits exceed limit")
                    for ci in range(0, len(extra), max_waits):
                        nop = mybir.InstNoOp(
                            name=f"{inst.name}_waitc{ci}",
                            engine=inst.engine,
                            bass_nofuse=True,
                            sync_info=mybir.SyncInfo(
                                on_wait=extra[ci:ci + max_waits], on_update=[]),
                        )
                        new_insts.append(nop)
                    si.on_wait = keep
                    inst.sync_info = si
                    changed = True
                new_insts.append(inst)
            if changed:
                bb.instructions = new_insts


def build(N_pad=N_PAD, NQ_=NQ, D_=D, R=1, drainfix=True):
    KT = N_pad // 128
    DT = D_ // 128
    QRT = NQ_ // 128
    qblocks = []
    q0 = 0
    while q0 < NQ_:
        qsz = min(512, NQ_ - q0)
        qblocks.append((q0, qsz))
        q0 += qsz

    nc = bass.Bass("TRN2", target_bir_lowering=False, debug=False,
                   num_devices=N_CORES)
    xb = nc.declare_dram_parameter("xb", [N_pad, D_], BF16, isOutput=False)
    xbT = nc.declare_dram_parameter("xbT", [D_, N_pad], BF16, isOutput=False)
    maskT = nc.declare_dram_parameter("maskT", [N_pad, NQ_], U8,
                                      isOutput=False)
    xq = nc.declare_dram_parameter("xq", [NQ_, D_], F32, isOutput=False)
    out = nc.declare_dram_parameter("out", [NQ_, D_], F32, isOutput=True)

    with tile.TileContext(nc) as tc, ExitStack() as ctx:
        resident = ctx.enter_context(tc.tile_pool(name="resident", bufs=1))
        small = ctx.enter_context(tc.tile_pool(name="small", bufs=1))
        sq_pool = ctx.enter_context(tc.tile_pool(name="sq", bufs=1))
        xq_pool = ctx.enter_context(tc.tile_pool(name="xqp", bufs=1))
        qn_pool = ctx.enter_context(tc.tile_pool(name="qnp", bufs=2))
        mask_pool = ctx.enter_context(tc.tile_pool(name="maskp", bufs=8))
        e_pool = ctx.enter_context(tc.tile_pool(name="ep", bufs=3))
        p_pool = ctx.enter_context(tc.tile_pool(name="pp", bufs=4))
        fin_pool = ctx.enter_context(tc.tile_pool(name="finp", bufs=1))
        sim_psum = ctx.enter_context(
            tc.tile_pool(name="simps", bufs=3, space="PSUM"))
        acc_psum = ctx.enter_context(
            tc.tile_pool(name="accps", bufs=1, space="PSUM"))

        # xbT column-chunking so the key loop can start before the full
        # transpose is resident (chunk g covers key tiles [g*CHK, (g+1)*CHK))
        CH = min(8, KT)
        CHK = KT // CH
        assert KT % CH == 0
        CHW = N_pad // CH

        def body(_i=None):
            xb_s = resident.tile([128, KT, D_], BF16, tag="xb_s")
            xbT_s = resident.tile([128, DT, N_pad], BF16, tag="xbT_s")

            identity = small.tile([128, 128], F32, tag="ident")
            make_identity(nc, identity)
            ones_b = small.tile([128, 1], BF16, tag="ones")
            nc.vector.memset(ones_b, 1.0)
            eps = small.tile([128, 1], F32, tag="eps")
            nc.vector.memset(eps, 1e-12)
            zero4 = small.tile([128, 4], BF16, tag="zero4")
            nc.vector.memset(zero4, 0.0)

            ssq = small.tile([128, KT], F32, tag="ssq")
            nrm = small.tile([128, KT], F32, tag="nrm")
            rn = small.tile([128, KT], F32, tag="rn")

            def load_xbT_chunk(g):
                for dt in range(DT):
                    nc.sync.dma_start(
                        out=xbT_s[:, dt, g * CHW:(g + 1) * CHW],
                        in_=xbT[dt * 128:(dt + 1) * 128,
                                g * CHW:(g + 1) * CHW])

            def load_xb_square(t):
                # load x rows for key tile t; Square shares the ACT table
                # with Exp, so this costs no table switch (GPSIMD+DVE variant
                # measured slower on HW: 594us vs 522us)
                nc.sync.dma_start(out=xb_s[:, t, :],
                                  in_=xb[t * 128:(t + 1) * 128, :])
                sq_scr = sq_pool.tile([128, D_], BF16, tag="sqscr")
                nc.scalar.activation(out=sq_scr, in_=xb_s[:, t, :],
                                     func=AF.Square,
                                     accum_out=ssq[:, t:t + 1])

            def norm_group(g):
                # one Sqrt per CHK tiles (each Sqrt costs 2 ACT table loads:
                # to sqrt_and_friends and back to exp_and_friends)
                sl = slice(g * CHK, (g + 1) * CHK)
                nc.scalar.activation(out=nrm[:, sl], in_=ssq[:, sl],
                                     func=AF.Sqrt, bias=eps)
                nc.vector.reciprocal(out=rn[:, sl], in_=nrm[:, sl])

            load_xbT_chunk(0)

            # ---- query prep: qnT[d, q] = (x_q/|x_q|)^T in bf16 ----
            ssq_q = small.tile([128, QRT], F32, tag="ssq_q")
            nrm_q = small.tile([128, QRT], F32, tag="nrm_q")
            rn_q = small.tile([128, QRT], F32, tag="rn_q")
            qnT = resident.tile([128, DT, NQ_], BF16, tag="qnT")
            xq_js = []
            for j in range(QRT):
                xq_t = xq_pool.tile([128, D_], F32, tag=f"xq_t{j % 4}",
                                    name=f"xq_t{j % 4}")
                nc.sync.dma_start(out=xq_t, in_=xq[j * 128:(j + 1) * 128, :])
                sq_scr = sq_pool.tile([128, D_], BF16, tag="sqscr")
                nc.scalar.activation(out=sq_scr, in_=xq_t, func=AF.Square,
                                     accum_out=ssq_q[:, j:j + 1])
                xq_js.append(xq_t)
                if j % 4 == 3 or j == QRT - 1:
                    # batch Sqrt over the last <=4 query row blocks
                    lo = (j // 4) * 4
                    nc.scalar.activation(out=nrm_q[:, lo:j + 1],
                                         in_=ssq_q[:, lo:j + 1],
                                         func=AF.Sqrt, bias=eps)
                    nc.vector.reciprocal(out=rn_q[:, lo:j + 1],
                                         in_=nrm_q[:, lo:j + 1])
                    for jj in range(lo, j + 1):
                        qn_t = qn_pool.tile([128, D_], F32, tag="qn_t")
                        nc.vector.tensor_scalar_mul(
                            out=qn_t, in0=xq_js[jj],
                            scalar1=rn_q[:, jj:jj + 1])
                        for dt in range(DT):
                            # shares simT tag/banks (sized to larger tile)
                            tps = sim_psum.tile([128, 128], F32, tag="simT",
                                                name="tps")
                            nc.tensor.transpose(
                                tps, qn_t[:, dt * 128:(dt + 1) * 128],
                                identity)
                            nc.vector.tensor_copy(
                                out=qnT[:, dt, jj * 128:(jj + 1) * 128],
                                in_=tps)

            # ---- main flash loop over (query block, key tile) ----
            # During the first query block, resident x tiles / xbT chunks /
            # per-tile key norms stream in just ahead of their first use, so
            # PE work starts ~10us in instead of after a ~100us bulk load.
            PF = 2 * CHK  # xb-tile prefetch distance during qb0 (2 groups)
            for qbi, (q0_, qsz) in enumerate(qblocks):
                if qbi == 0:
                    for t in range(min(PF, KT)):
                        load_xb_square(t)
                    norm_group(0)
                    if CH > 1:
                        norm_group(1)
                nsub = (qsz + 127) // 128
                outp = [acc_psum.tile([128, D_], F32, tag=f"outp{j}",
                                      name=f"outp{j}") for j in range(nsub)]
                den = acc_psum.tile([128, 4], F32, tag="den")
                # start=True zeroes the full PSUM bank -> clear den's bank
                # once; per-column accumulations then use start=False.
                nc.tensor.matmul(den[:, 0:4], lhsT=xbT_s[:, 0, 0:128],
                                 rhs=zero4, start=True, stop=False,
                                 skip_group_check=True)
                for kt in range(KT):
                    if qbi == 0 and kt % CHK == CHK // 2 and kt // CHK + 1 < CH:
                        load_xbT_chunk(kt // CHK + 1)
                    simT = sim_psum.tile([128, 512], F32, tag="simT")
                    for dt in range(DT):
                        nc.tensor.matmul(
                            simT[:, :qsz],
                            lhsT=xbT_s[:, dt, kt * 128:(kt + 1) * 128],
                            rhs=qnT[:, dt, q0_:q0_ + qsz],
                            start=(dt == 0), stop=(dt == DT - 1))
                    e_t = e_pool.tile([128, 512], BF16, tag="e_t")
                    nc.scalar.activation(out=e_t[:, :qsz], in_=simT[:, :qsz],
                                         func=AF.Exp, scale=rn[:, kt:kt + 1])
                    m_t = mask_pool.tile([128, 512], U8, tag="m_t")
                    nc.sync.dma_start(
                        out=m_t[:, :qsz],
                        in_=maskT[kt * 128:(kt + 1) * 128, q0_:q0_ + qsz])
                    p_t = p_pool.tile([128, 512], BF16, tag="p_t")
                    nc.vector.tensor_tensor(out=p_t[:, :qsz], in0=e_t[:, :qsz],
                                            in1=m_t[:, :qsz], op=ALU.mult)
                    for j in range(nsub):
                        jsz = min(128, qsz - j * 128)
                        psl = p_t[:, j * 128:j * 128 + jsz]
                        nc.tensor.matmul(outp[j][:jsz, :], lhsT=psl,
                                         rhs=xb_s[:, kt, :],
                                         start=(kt == 0), stop=(kt == KT - 1))
                        nc.tensor.matmul(den[:jsz, j:j + 1], lhsT=psl,
                                         rhs=ones_b, start=False,
                                         stop=(kt == KT - 1),
                                         skip_group_check=True)
                    if qbi == 0 and kt + PF < KT:
                        # emitted after this iteration's exp so the prefetch
                        # square sits behind it in the ACT FIFO
                        load_xb_square(kt + PF)
                    if qbi == 0 and kt % CHK == CHK - 1 and kt // CHK + 2 < CH:
                        norm_group(kt // CHK + 2)
                # ---- finalize: out = 2*xq - outp/den ----
                for j in range(nsub):
                    jsz = min(128, qsz - j * 128)
                    r0 = q0_ + j * 128
                    rden = small.tile([128, 1], F32, tag="rden")
                    nc.vector.reciprocal(out=rden[:jsz],
                                         in_=den[:jsz, j:j + 1])
                    t1 = fin_pool.tile([128, D_], F32, tag="t1")
                    nc.vector.tensor_scalar_mul(out=t1[:jsz],
                                                in0=outp[j][:jsz, :],
                                                scalar1=rden[:jsz])
                    xq_f = fin_pool.tile([128, D_], F32, tag="xq_f")
                    nc.sync.dma_start(out=xq_f[:jsz], in_=xq[r0:r0 + jsz, :])
                    xq2 = fin_pool.tile([128, D_], F32, tag="xq2")
                    nc.scalar.mul(out=xq2[:jsz], in_=xq_f[:jsz], mul=2.0)
                    o_t = fin_pool.tile([128, D_], F32, tag="o_t")
                    nc.vector.tensor_tensor(out=o_t[:jsz], in0=xq2[:jsz],
                                            in1=t1[:jsz], op=ALU.subtract)
                    nc.sync.dma_start(out=out[r0:r0 + jsz, :], in_=o_t[:jsz])

        if R == 1:
            body()
        else:
            with tc.For_i(0, R, 1) as i:
                body(i)

    if drainfix:
        _split_excess_waits(nc, 1)
    return nc


def prep_inputs(x, adj):
    """Host-side shard/layout prep. Returns in_maps for run_bass_kernel_spmd."""
    bf16 = ml_dtypes.bfloat16
    xb = np.zeros((N_PAD, D), dtype=bf16)
    xb[:N] = x.astype(bf16)
    xbT = np.ascontiguousarray(xb.T)
    in_maps = []
    for c in range(N_CORES):
        q0 = c * NQ
        q1 = min(q0 + NQ, N)
        nreal = max(q1 - q0, 0)
        maskT_c = np.ones((N_PAD, NQ), dtype=np.uint8)
        if nreal > 0:
            maskT_c[:N, :nreal] = (1 - adj[q0:q1, :].T).astype(np.uint8)
            maskT_c[N:, :nreal] = 0
        xq_c = np.zeros((NQ, D), dtype=np.float32)
        if nreal > 0:
            xq_c[:nreal] = x[q0:q1]
        in_maps.append({"xb": xb, "xbT": xbT, "maskT": maskT_c, "xq": xq_c})
    return in_maps


_cached = {}


def _get_nc(R=1):
    if R not in _cached:
        _cached[R] = build(R=R)
    return _cached[R]


_neff_cache_installed = False


def _install_neff_cache():
    """Disk-cache walrus NEFF compiles keyed by the BIR JSON hash, so repeat
    processes skip the multi-minute compile."""
    global _neff_cache_installed
    if _neff_cache_installed:
        return
    _neff_cache_installed = True
    import hashlib
    import shutil
    from concourse import bass2jax
    cache_dir = os.path.expanduser("~/.cache/bass_neff_cache")
    os.makedirs(cache_dir, exist_ok=True)
    orig = bass2jax.compile_bir_kernel

    def cached(bir_json, tmpdir, neff_name="file.neff"):
        key = hashlib.sha256(
            bir_json if isinstance(bir_json, bytes) else bir_json.encode()
        ).hexdigest()[:32]
        hit = os.path.join(cache_dir, key + ".neff")
        dst = os.path.join(tmpdir, neff_name)
        if os.path.exists(hit):
            shutil.copyfile(hit, dst)
            return dst
        path = orig(bir_json, tmpdir, neff_name)
        try:
            shutil.copyfile(path, hit)
        except OSError:
            pass
        return path

    bass2jax.compile_bir_kernel = cached


def run_on_cores(in_maps, R=1):
    _install_neff_cache()
    from concourse.bass_utils import run_bass_kernel_spmd
    nc = _get_nc(R)
    res = run_bass_kernel_spmd(nc, in_maps, list(range(N_CORES)))
    return [res.results[c]["out"] for c in range(N_CORES)]


def kernel(x, adj):
    x = np.asarray(x, dtype=np.float32)
    adj = np.asarray(adj, dtype=np.int32)
    assert x.shape == (N, D) and adj.shape == (N, N)
    in_maps = prep_inputs(x, adj)
    outs = run_on_cores(in_maps, R=1)
    full = np.concatenate(outs, axis=0)[:N]
    return np.ascontiguousarray(full.astype(np.float32))



# revision 13
# speedup vs baseline: 1.9582x; 1.9582x over previous
"""ContraNorm (NormLayer 'CN' branch) on 8 Trainium2 NeuronCores.

kernel(x, adj) -> (1+s)*x - s * softmax(mask(cossim(x, x))) @ x  with s=1.

Strategy: shard queries (rows) across the 8 cores (1280 rows each, N padded
10000 -> 10240); every core holds all keys.  Both big matmuls run in fp8-e4m3
with MatmulPerfMode.DoubleRow (256-deep contraction per instruction, 2x the
bf16 PE rate):

  simT[k, q] = sum_d x8T[d, k] * qn8T[d, q]   (2 DoubleRow matmuls per key
                                               tile, pairing d-halves)
  outp[q, d] = sum_k p8[k, q] * x8[k, d]      (1 DoubleRow matmul per key
                                               tile PAIR per 128-query block)

Key inverse norms (1/|x8_k|, folded into exp as a per-partition activation
scale) and the normalized/quantized/folded query matrix qn8T are precomputed
on host, so the device does no Square/Sqrt/transpose prep at all -- the ACT
engine only ever runs Exp (no table switches).  Cosines lie in [-1, 1] so exp
never overflows and no max-subtraction pass is needed.  The (1-adj)^T u8 mask
multiplies exp's fp8 output (exact zeros, matching -inf softmax); the softmax
denominator accumulates in PSUM via DoubleRow matmuls against a ones vector.
"""
import os
import sys

sys.path.insert(0, '/opt/trn_rl_repo')

from contextlib import ExitStack

import numpy as np
import ml_dtypes

import concourse.bass as bass
import concourse.tile as tile
from concourse import mybir
from concourse.masks import make_identity

F32 = mybir.dt.float32
F8 = mybir.dt.float8e4
U8 = mybir.dt.uint8
AF = mybir.ActivationFunctionType
ALU = mybir.AluOpType
DR = mybir.MatmulPerfMode.DoubleRow

N = 10000
D = 512
N_CORES = 8
N_PAD = 10240            # 80 key tiles of 128; == 8 * 1280
NQ = N_PAD // N_CORES    # 1280 query rows per core (core 7: 1040 real)


def _split_excess_waits(nc, max_waits=1):
    """Walrus CoreV3 rejects >1 sync wait per CTRL instruction; Tile's tail
    drain carries one wait per outstanding engine/DMA queue.  Hoist monotone
    (sem-ge) waits onto same-engine NoOps placed immediately before the
    offending instruction — semantically identical, since the engine executes
    them in program order."""
    for f in nc.m.functions:
        for bb in f.blocks:
            insts = list(bb.instructions)
            new_insts = []
            changed = False
            for inst in insts:
                si = inst.sync_info
                waits = list(si.on_wait) if si is not None else []
                if len(waits) > max_waits:
                    ge = [w for w in waits if "eq" not in (w.wait_mode or "")]
                    eq = [w for w in waits if "eq" in (w.wait_mode or "")]
                    keep_n = max(max_waits - len(eq), 0)
                    n_extra = max(len(ge) - keep_n, 0)
                    extra, keep = ge[:n_extra], ge[n_extra:] + eq
                    if len(keep) > max_waits:
                        raise RuntimeError(
                            f"{inst.name}: non-monotone waits exceed limit")
                    for ci in range(0, len(extra), max_waits):
                        nop = mybir.InstNoOp(
                            name=f"{inst.name}_waitc{ci}",
                            engine=inst.engine,
                            bass_nofuse=True,
                            sync_info=mybir.SyncInfo(
                                on_wait=extra[ci:ci + max_waits], on_update=[]),
                        )
                        new_insts.append(nop)
                    si.on_wait = keep
                    inst.sync_info = si
                    changed = True
                new_insts.append(inst)
            if changed:
                bb.instructions = new_insts


def build(N_pad=N_PAD, NQ_=NQ, D_=D, R=1, drainfix=True):
    KT = N_pad // 128        # 80 key tiles
    KT2 = KT // 2            # 40 key tile pairs (DoubleRow pv contraction)
    DT = D_ // 128           # 4 d-groups of 128 (pairs 2*di, 2*di+1 for sim)
    qblocks = []
    q0 = 0
    while q0 < NQ_:
        qsz = min(512, NQ_ - q0)
        qblocks.append((q0, qsz))
        q0 += qsz

    nc = bass.Bass("TRN2", target_bir_lowering=False, debug=False,
                   num_devices=N_CORES)
    xb = nc.declare_dram_parameter("xb", [N_pad, D_], F8, isOutput=False)
    xbT = nc.declare_dram_parameter("xbT", [128, DT, N_pad], F8,
                                    isOutput=False)
    qnT = nc.declare_dram_parameter("qnT", [128, DT, NQ_], F8, isOutput=False)
    rnp = nc.declare_dram_parameter("rn", [128, KT], F32, isOutput=False)
    maskT = nc.declare_dram_parameter("maskT", [N_pad, NQ_], U8,
                                      isOutput=False)
    xq2 = nc.declare_dram_parameter("xq2", [NQ_, D_], F32, isOutput=False)
    out = nc.declare_dram_parameter("out", [NQ_, D_], F32, isOutput=True)

    with tile.TileContext(nc) as tc, ExitStack() as ctx:
        resident = ctx.enter_context(tc.tile_pool(name="resident", bufs=1))
        small = ctx.enter_context(tc.tile_pool(name="small", bufs=1))
        mask_pool = ctx.enter_context(tc.tile_pool(name="maskp", bufs=8))
        e_pool = ctx.enter_context(tc.tile_pool(name="ep", bufs=3))
        p_pool = ctx.enter_context(tc.tile_pool(name="pp", bufs=3))
        x2_pool = ctx.enter_context(tc.tile_pool(name="x2p", bufs=4))
        fin_pool = ctx.enter_context(tc.tile_pool(name="finp", bufs=2))
        sim_psum = ctx.enter_context(
            tc.tile_pool(name="simps", bufs=3, space="PSUM"))
        acc_psum = ctx.enter_context(
            tc.tile_pool(name="accps", bufs=1, space="PSUM"))

        # xbT column-chunking so the key loop can start before the full
        # transpose is resident (chunk g covers key tiles [g*CHK, (g+1)*CHK))
        CH = min(8, KT)
        CHK = KT // CH
        assert KT % CH == 0
        CHW = N_pad // CH

        def body(_i=None):
            xb_s = resident.tile([128, KT, D_], F8, tag="xb_s")
            xbT_s = resident.tile([128, DT, N_pad], F8, tag="xbT_s")
            qnT_s = resident.tile([128, DT, NQ_], F8, tag="qnT_s")
            rn = small.tile([128, KT], F32, tag="rn")

            # 32 identical ones columns: a 1-column DoubleRow ldweights fails
            # walrus's ISA check (min tile col size 32); the 32 resulting
            # identical den rows cost nothing extra (same moving stream)
            ones2 = small.tile([128, 2, 32], F8, tag="ones2")
            nc.vector.memset(ones2, 1.0)
            identity = small.tile([128, 128], F32, tag="ident")
            make_identity(nc, identity)

            nc.sync.dma_start(out=rn, in_=rnp[:, :])
            for i in range(DT):
                nc.sync.dma_start(out=qnT_s[:, i, :], in_=qnT[:, i, :])

            def load_xbT_chunk(g):
                for i in range(DT):
                    nc.sync.dma_start(
                        out=xbT_s[:, i, g * CHW:(g + 1) * CHW],
                        in_=xbT[:, i, g * CHW:(g + 1) * CHW])

            def load_xb(t):
                nc.sync.dma_start(out=xb_s[:, t, :],
                                  in_=xb[t * 128:(t + 1) * 128, :])

            load_xbT_chunk(0)

            # ---- main flash loop over (query block, key tile) ----
            # During the first query block, resident xb tiles / xbT chunks
            # stream in just ahead of their first use, so PE work starts a
            # few us in instead of after a ~30us bulk load.
            PF = 2 * CHK  # xb-tile prefetch distance during qb0 (2 chunks)
            for qbi, (q0_, qsz) in enumerate(qblocks):
                if qbi == 0:
                    for t in range(min(PF, KT)):
                        load_xb(t)
                nsub = (qsz + 127) // 128
                outp = [acc_psum.tile([128, D_], F32, tag=f"outp{j}",
                                      name=f"outp{j}") for j in range(nsub)]
                # softmax denominator as a [1, qsz] row: one DoubleRow matmul
                # per key-tile pair with the constant ones vector stationary
                # (2-column weight load hides under any neighbor, unlike a
                # per-j 1-column matmul whose successor's 128-column weight
                # load cannot hide behind its 1-cycle compute)
                denr = acc_psum.tile([32, 512], F32, tag="denr")
                # prefetch this block's xq2 rows; DMA overlaps the kt loop
                xq2_js = []
                for j in range(nsub):
                    jsz = min(128, qsz - j * 128)
                    xq2_t = x2_pool.tile([128, D_], F32, tag=f"xq2_{j}",
                                         name=f"xq2_{j}")
                    nc.sync.dma_start(
                        out=xq2_t[:jsz],
                        in_=xq2[q0_ + j * 128:q0_ + j * 128 + jsz, :])
                    xq2_js.append(xq2_t)
                p2 = None
                for kt in range(KT):
                    if qbi == 0 and kt % CHK == CHK // 2 and kt // CHK + 1 < CH:
                        load_xbT_chunk(kt // CHK + 1)
                    kk = kt % 2
                    if kk == 0:
                        p2 = p_pool.tile([128, 2, 512], F8, tag="p2")
                    simT = sim_psum.tile([128, 512], F32, tag="simT")
                    for di in range(DT // 2):
                        nc.tensor.matmul(
                            simT[:, :qsz],
                            lhsT=xbT_s[:, 2 * di:2 * di + 2,
                                       kt * 128:(kt + 1) * 128],
                            rhs=qnT_s[:, 2 * di:2 * di + 2, q0_:q0_ + qsz],
                            start=(di == 0), stop=(di == DT // 2 - 1),
                            perf_mode=DR)
                    e_t = e_pool.tile([128, 512], F8, tag="e_t")
                    nc.scalar.activation(out=e_t[:, :qsz], in_=simT[:, :qsz],
                                         func=AF.Exp, scale=rn[:, kt:kt + 1])
                    m_t = mask_pool.tile([128, 512], U8, tag="m_t")
                    nc.sync.dma_start(
                        out=m_t[:, :qsz],
                        in_=maskT[kt * 128:(kt + 1) * 128, q0_:q0_ + qsz])
                    nc.vector.tensor_tensor(out=p2[:, kk, :qsz],
                                            in0=e_t[:, :qsz],
                                            in1=m_t[:, :qsz], op=ALU.mult)
                    if kk == 1:
                        kt2 = kt // 2
                        for j in range(nsub):
                            jsz = min(128, qsz - j * 128)
                            psl = p2[:, :, j * 128:j * 128 + jsz]
                            nc.tensor.matmul(
                                outp[j][:jsz, :], lhsT=psl,
                                rhs=xb_s[:, kt - 1:kt + 1, :],
                                start=(kt2 == 0), stop=(kt2 == KT2 - 1),
                                perf_mode=DR)
                        nc.tensor.matmul(
                            denr[:32, :qsz], lhsT=ones2, rhs=p2[:, :, :qsz],
                            start=(kt2 == 0), stop=(kt2 == KT2 - 1),
                            perf_mode=DR)
                    if qbi == 0 and kt + PF < KT:
                        load_xb(kt + PF)
                # ---- finalize: out = xq2 - outp/den ----
                rdrow = small.tile([32, 512], F32, tag="rdrow")
                nc.vector.reciprocal(out=rdrow[:, :qsz], in_=denr[:32, :qsz])
                for j in range(nsub):
                    jsz = min(128, qsz - j * 128)
                    r0 = q0_ + j * 128
                    rd_ps = sim_psum.tile([128, 512], F32, tag="simT",
                                          name="rd_ps")
                    nc.tensor.transpose(
                        rd_ps[:jsz, 0:32],
                        rdrow[:, j * 128:j * 128 + jsz], identity[0:32, 0:32])
                    t1 = fin_pool.tile([128, D_], F32, tag="t1")
                    nc.vector.tensor_scalar_mul(out=t1[:jsz],
                                                in0=outp[j][:jsz, :],
                                                scalar1=rd_ps[:jsz, 0:1])
                    o_t = fin_pool.tile([128, D_], F32, tag="o_t")
                    nc.vector.tensor_tensor(out=o_t[:jsz],
                                            in0=xq2_js[j][:jsz],
                                            in1=t1[:jsz], op=ALU.subtract)
                    nc.sync.dma_start(out=out[r0:r0 + jsz, :], in_=o_t[:jsz])

        if R == 1:
            body()
        else:
            with tc.For_i(0, R, 1) as i:
                body(i)

    if drainfix:
        _split_excess_waits(nc, 1)
    return nc


def prep_inputs(x, adj):
    """Host-side shard/layout prep. Returns in_maps for run_bass_kernel_spmd."""
    f8 = ml_dtypes.float8_e4m3
    DT = D // 128
    x8 = np.zeros((N_PAD, D), dtype=f8)
    x8[:N] = x.astype(f8)
    x8f = x8.astype(np.float32)
    # key inverse norms of the QUANTIZED keys (consistent with the fp8
    # matmul), folded into exp as its per-partition scale
    rn = np.zeros((N_PAD,), dtype=np.float32)
    nrm = np.sqrt((x8f[:N] ** 2).sum(axis=1))
    rn[:N] = 1.0 / np.maximum(nrm, 1e-12)
    rn_s = np.ascontiguousarray(rn.reshape(KT_ := N_PAD // 128, 128).T)
    # folded transpose [128, DT, N_PAD]: xbT[p, i, n] = x8[n, i*128+p]
    xbT = np.ascontiguousarray(
        x8.T.reshape(DT, 128, N_PAD).transpose(1, 0, 2))
    # normalized queries from the fp32 x, then quantized + folded
    qn = x / np.maximum(np.linalg.norm(x, axis=1, keepdims=True), 1e-12)
    qn8 = np.zeros((N_PAD, D), dtype=f8)
    qn8[:N] = qn.astype(f8)
    in_maps = []
    for c in range(N_CORES):
        q0 = c * NQ
        q1 = min(q0 + NQ, N)
        nreal = max(q1 - q0, 0)
        maskT_c = np.ones((N_PAD, NQ), dtype=np.uint8)
        if nreal > 0:
            maskT_c[:N, :nreal] = (1 - adj[q0:q1, :].T).astype(np.uint8)
            maskT_c[N:, :nreal] = 0
        qnT_c = np.ascontiguousarray(
            qn8[q0:q0 + NQ].T.reshape(DT, 128, NQ).transpose(1, 0, 2))
        xq2_c = np.zeros((NQ, D), dtype=np.float32)
        if nreal > 0:
            xq2_c[:nreal] = 2.0 * x[q0:q1]
        in_maps.append({"xb": x8, "xbT": xbT, "qnT": qnT_c, "rn": rn_s,
                        "maskT": maskT_c, "xq2": xq2_c})
    return in_maps


_cached = {}


def _get_nc(R=1):
    if R not in _cached:
        _cached[R] = build(R=R)
    return _cached[R]


_neff_cache_installed = False


def _install_neff_cache():
    """Disk-cache walrus NEFF compiles keyed by the BIR JSON hash, so repeat
    processes skip the multi-minute compile."""
    global _neff_cache_installed
    if _neff_cache_installed:
        return
    _neff_cache_installed = True
    import hashlib
    import shutil
    from concourse import bass2jax
    cache_dir = os.path.expanduser("~/.cache/bass_neff_cache")
    os.makedirs(cache_dir, exist_ok=True)
    orig = bass2jax.compile_bir_kernel

    def cached(bir_json, tmpdir, neff_name="file.neff"):
        key = hashlib.sha256(
            bir_json if isinstance(bir_json, bytes) else bir_json.encode()
        ).hexdigest()[:32]
        hit = os.path.join(cache_dir, key + ".neff")
        dst = os.path.join(tmpdir, neff_name)
        if os.path.exists(hit):
            shutil.copyfile(hit, dst)
            return dst
        path = orig(bir_json, tmpdir, neff_name)
        try:
            shutil.copyfile(path, hit)
        except OSError:
            pass
        return path

    bass2jax.compile_bir_kernel = cached


def run_on_cores(in_maps, R=1):
    _install_neff_cache()
    from concourse.bass_utils import run_bass_kernel_spmd
    nc = _get_nc(R)
    res = run_bass_kernel_spmd(nc, in_maps, list(range(N_CORES)))
    return [res.results[c]["out"] for c in range(N_CORES)]


def kernel(x, adj):
    x = np.asarray(x, dtype=np.float32)
    adj = np.asarray(adj, dtype=np.int32)
    assert x.shape == (N, D) and adj.shape == (N, N)
    in_maps = prep_inputs(x, adj)
    outs = run_on_cores(in_maps, R=1)
    full = np.concatenate(outs, axis=0)[:N]
    return np.ascontiguousarray(full.astype(np.float32))


# revision 30
# speedup vs baseline: 2.4250x; 1.2384x over previous
"""ContraNorm (NormLayer 'CN' branch) on 8 Trainium2 NeuronCores.

kernel(x, adj) -> (1+s)*x - s * softmax(mask(cossim(x, x))) @ x  with s=1.

Strategy: shard queries (rows) across the 8 cores (1280 rows each, N padded
10000 -> 10240); every core holds all keys.  Both big matmuls run in fp8-e4m3
with MatmulPerfMode.DoubleRow (256-deep contraction per instruction, 2x the
bf16 PE rate):

  simT[k, q] = sum_d x8T[d, k] * qn8T[d, q]   (2 DoubleRow matmuls per key
                                               tile, pairing d-halves)
  outp[q, d] = sum_k p8[k, q] * x8[k, d]      (1 DoubleRow matmul per key
                                               tile PAIR per 128-query block)

Key inverse norms (1/|x8_k|, folded into exp as a per-partition activation
scale) and the normalized/quantized/folded query matrix qn8T are precomputed
on host, so the device does no Square/Sqrt/transpose prep at all -- the ACT
engine only ever runs Exp (no table switches).  Cosines lie in [-1, 1] so exp
never overflows and no max-subtraction pass is needed.  The (1-adj)^T u8 mask
multiplies exp's fp8 output (exact zeros, matching -inf softmax); the softmax
denominator accumulates in PSUM via DoubleRow matmuls against a ones vector.
"""
import os
import sys

sys.path.insert(0, '/opt/trn_rl_repo')

from contextlib import ExitStack

import numpy as np
import ml_dtypes

import concourse.bass as bass
import concourse.tile as tile
from concourse import mybir

F32 = mybir.dt.float32
F8 = mybir.dt.float8e4
U8 = mybir.dt.uint8
AF = mybir.ActivationFunctionType
ALU = mybir.AluOpType
DR = mybir.MatmulPerfMode.DoubleRow

N = 10000
D = 512
N_CORES = 8
N_PAD = 10240            # 80 key tiles of 128; == 8 * 1280
NQ = N_PAD // N_CORES    # 1280 query rows per core (core 7: 1040 real)


def _split_excess_waits(nc, max_waits=1):
    """Walrus CoreV3 rejects >1 sync wait per CTRL instruction; Tile's tail
    drain carries one wait per outstanding engine/DMA queue.  Hoist monotone
    (sem-ge) waits onto same-engine NoOps placed immediately before the
    offending instruction — semantically identical, since the engine executes
    them in program order."""
    for f in nc.m.functions:
        for bb in f.blocks:
            insts = list(bb.instructions)
            new_insts = []
            changed = False
            for inst in insts:
                si = inst.sync_info
                waits = list(si.on_wait) if si is not None else []
                if len(waits) > max_waits:
                    ge = [w for w in waits if "eq" not in (w.wait_mode or "")]
                    eq = [w for w in waits if "eq" in (w.wait_mode or "")]
                    keep_n = max(max_waits - len(eq), 0)
                    n_extra = max(len(ge) - keep_n, 0)
                    extra, keep = ge[:n_extra], ge[n_extra:] + eq
                    if len(keep) > max_waits:
                        raise RuntimeError(
                            f"{inst.name}: non-monotone waits exceed limit")
                    for ci in range(0, len(extra), max_waits):
                        nop = mybir.InstNoOp(
                            name=f"{inst.name}_waitc{ci}",
                            engine=inst.engine,
                            bass_nofuse=True,
                            sync_info=mybir.SyncInfo(
                                on_wait=extra[ci:ci + max_waits], on_update=[]),
                        )
                        new_insts.append(nop)
                    si.on_wait = keep
                    inst.sync_info = si
                    changed = True
                new_insts.append(inst)
            if changed:
                bb.instructions = new_insts


def build(N_pad=N_PAD, NQ_=NQ, D_=D, R=1, drainfix=True):
    KT = N_pad // 128        # 80 key tiles
    KT2 = KT // 2            # 40 key tile pairs (DoubleRow pv contraction)
    DT = D_ // 128           # 4 d-groups of 128 (pairs 2*di, 2*di+1 for sim)
    qblocks = []
    q0 = 0
    while q0 < NQ_:
        qsz = min(512, NQ_ - q0)
        qblocks.append((q0, qsz))
        q0 += qsz

    nc = bass.Bass("TRN2", target_bir_lowering=False, debug=False,
                   num_devices=N_CORES)
    # key-axis tensors are host-folded to [128, KT, .] so multi-tile DMAs
    # linearize correctly (dst [p, t, q] <- src [p, t, q], same dim order)
    xb = nc.declare_dram_parameter("xb", [128, KT, D_], F8, isOutput=False)
    xbT = nc.declare_dram_parameter("xbT", [128, DT, N_pad], F8,
                                    isOutput=False)
    qnT = nc.declare_dram_parameter("qnT", [128, DT, NQ_], F8, isOutput=False)
    rnp = nc.declare_dram_parameter("rn", [128, KT], F32, isOutput=False)
    maskT = nc.declare_dram_parameter("maskT", [128, KT, NQ_], U8,
                                      isOutput=False)
    xq2 = nc.declare_dram_parameter("xq2", [NQ_, D_], F32, isOutput=False)
    out = nc.declare_dram_parameter("out", [NQ_, D_], F32, isOutput=True)

    with tile.TileContext(nc) as tc, ExitStack() as ctx:
        resident = ctx.enter_context(tc.tile_pool(name="resident", bufs=1))
        small = ctx.enter_context(tc.tile_pool(name="small", bufs=1))
        mask_pool = ctx.enter_context(tc.tile_pool(name="maskp", bufs=8))
        e_pool = ctx.enter_context(tc.tile_pool(name="ep", bufs=3))
        p_pool = ctx.enter_context(tc.tile_pool(name="pp", bufs=3))
        x2_pool = ctx.enter_context(tc.tile_pool(name="x2p", bufs=4))
        fin_pool = ctx.enter_context(tc.tile_pool(name="finp", bufs=2))
        sim_psum = ctx.enter_context(
            tc.tile_pool(name="simps", bufs=3, space="PSUM"))
        acc_psum = ctx.enter_context(
            tc.tile_pool(name="accps", bufs=1, space="PSUM"))

        # xbT column-chunking so the key loop can start before the full
        # transpose is resident (chunk g covers key tiles [g*CHK, (g+1)*CHK))
        CH = min(8, KT)
        CHK = KT // CH
        assert KT % CH == 0
        CHW = N_pad // CH

        def body(_i=None):
            xb_s = resident.tile([128, KT, D_], F8, tag="xb_s")
            xbT_s = resident.tile([128, DT, N_pad], F8, tag="xbT_s")
            qnT_s = resident.tile([128, DT, NQ_], F8, tag="qnT_s")
            rn = small.tile([128, KT], F32, tag="rn")

            ones2 = small.tile([128, 2, 1], F8, tag="ones2")
            nc.vector.memset(ones2, 1.0)
            zero4 = small.tile([128, 4], F8, tag="zero4")
            nc.vector.memset(zero4, 0.0)
            zjunk = small.tile([128, 128], F8, tag="zjunk")
            nc.vector.memset(zjunk, 0.0)

            nc.sync.dma_start(out=rn, in_=rnp[:, :])
            # qb0's query columns first so the first sim matmul isn't
            # gated on the full qnT load
            for q0_, qsz in qblocks:
                nc.sync.dma_start(out=qnT_s[:, :, q0_:q0_ + qsz],
                                  in_=qnT[:, :, q0_:q0_ + qsz])

            def load_xbT_chunk(g):
                nc.sync.dma_start(
                    out=xbT_s[:, :, g * CHW:(g + 1) * CHW],
                    in_=xbT[:, :, g * CHW:(g + 1) * CHW])

            XG = 4  # xb tiles per DMA (2KB contiguous per partition)

            def load_xb_group(g):
                nc.sync.dma_start(out=xb_s[:, g * XG:(g + 1) * XG, :],
                                  in_=xb[:, g * XG:(g + 1) * XG, :])

            load_xbT_chunk(0)

            # ---- main flash loop over (query block, key tile) ----
            # During the first query block, resident xb tiles / xbT chunks
            # stream in just ahead of their first use, so PE work starts a
            # few us in instead of after a ~30us bulk load.
            PFG = 5  # xb-group prefetch distance during qb0 (20 tiles)
            for qbi, (q0_, qsz) in enumerate(qblocks):
                if qbi == 0:
                    for g in range(PFG):
                        load_xb_group(g)
                nsub = (qsz + 127) // 128
                outp = [acc_psum.tile([128, D_], F32, tag=f"outp{j}",
                                      name=f"outp{j}") for j in range(nsub)]
                # per-j den matmuls reuse the pv matmul's just-loaded lhsT
                # (identical weights AP -> no reload), so they are near-free;
                # a ones-stationary [1, qsz] row variant measured 41us SLOWER
                # (it adds a real 512-row moving stream per pair)
                den = acc_psum.tile([128, 4], F32, tag="den")
                # start=True zeroes the full PSUM bank -> clear den's bank
                # once; per-column accumulations then use start=False.
                nc.tensor.matmul(den[:, 0:4], lhsT=zjunk, rhs=zero4,
                                 start=True, stop=False, skip_group_check=True)
                # prefetch this block's xq2 rows; DMA overlaps the kt loop
                xq2_js = []
                for j in range(nsub):
                    jsz = min(128, qsz - j * 128)
                    xq2_t = x2_pool.tile([128, D_], F32, tag=f"xq2_{j}",
                                         name=f"xq2_{j}")
                    nc.sync.dma_start(
                        out=xq2_t[:jsz],
                        in_=xq2[q0_ + j * 128:q0_ + j * 128 + jsz, :])
                    xq2_js.append(xq2_t)
                p2 = None
                e2 = None
                # kt 79 is pure padding (rows 10112..10239, mask all 0):
                # skip its sim/exp/mask work; p2 slot 1 of the last pair is
                # memset to 0 instead (xb tile 79 is zero-filled from host)
                for kt in range(KT - 1):
                    if qbi == 0 and kt % CHK == CHK // 2 and kt // CHK + 1 < CH:
                        load_xbT_chunk(kt // CHK + 1)
                    kk = kt % 2
                    if kk == 0:
                        p2 = p_pool.tile([128, 2, 512], F8, tag="p2")
                        e2 = e_pool.tile([128, 2, 512], F8, tag="e2")
                        m2 = mask_pool.tile([128, 2, 512], U8, tag="m2")
                        nslot = 1 if kt == KT - 2 else 2
                        nc.sync.dma_start(
                            out=m2[:, 0:nslot, :qsz],
                            in_=maskT[:, kt:kt + nslot, q0_:q0_ + qsz])
                    simT = sim_psum.tile([128, 512], F32, tag="simT")
                    for di in range(DT // 2):
                        nc.tensor.matmul(
                            simT[:, :qsz],
                            lhsT=xbT_s[:, 2 * di:2 * di + 2,
                                       kt * 128:(kt + 1) * 128],
                            rhs=qnT_s[:, 2 * di:2 * di + 2, q0_:q0_ + qsz],
                            start=(di == 0), stop=(di == DT // 2 - 1),
                            perf_mode=DR)
                    nc.scalar.activation(out=e2[:, kk, :qsz],
                                         in_=simT[:, :qsz],
                                         func=AF.Exp, scale=rn[:, kt:kt + 1])
                    if kt == KT - 2:
                        nc.vector.tensor_tensor(out=p2[:, 0, :qsz],
                                                in0=e2[:, 0, :qsz],
                                                in1=m2[:, 0, :qsz],
                                                op=ALU.mult)
                        nc.vector.memset(p2[:, 1, :qsz], 0.0)
                    elif kk == 1:
                        nc.vector.tensor_tensor(out=p2[:, :, :qsz],
                                                in0=e2[:, :, :qsz],
                                                in1=m2[:, :, :qsz],
                                                op=ALU.mult)
                    if kk == 1 or kt == KT - 2:
                        kb = kt - kk   # even tile of this pair
                        kt2 = kb // 2
                        for j in range(nsub):
                            jsz = min(128, qsz - j * 128)
                            psl = p2[:, :, j * 128:j * 128 + jsz]
                            nc.tensor.matmul(
                                outp[j][:jsz, :], lhsT=psl,
                                rhs=xb_s[:, kb:kb + 2, :],
                                start=(kt2 == 0), stop=(kt2 == KT2 - 1),
                                perf_mode=DR)
                            nc.tensor.matmul(
                                den[:jsz, j:j + 1], lhsT=psl, rhs=ones2,
                                start=False, stop=(kt2 == KT2 - 1),
                                perf_mode=DR, skip_group_check=True)
                    if qbi == 0 and kt % XG == 0 and \
                            (kt // XG) + PFG < KT // XG:
                        load_xb_group(kt // XG + PFG)
                # ---- finalize: out = xq2 - outp/den ----
                for j in range(nsub):
                    jsz = min(128, qsz - j * 128)
                    r0 = q0_ + j * 128
                    rden = small.tile([128, 1], F32, tag="rden")
                    nc.vector.reciprocal(out=rden[:jsz],
                                         in_=den[:jsz, j:j + 1])
                    t1 = fin_pool.tile([128, D_], F32, tag="t1")
                    nc.vector.tensor_scalar_mul(out=t1[:jsz],
                                                in0=outp[j][:jsz, :],
                                                scalar1=rden[:jsz])
                    o_t = fin_pool.tile([128, D_], F32, tag="o_t")
                    nc.vector.tensor_tensor(out=o_t[:jsz],
                                            in0=xq2_js[j][:jsz],
                                            in1=t1[:jsz], op=ALU.subtract)
                    nc.sync.dma_start(out=out[r0:r0 + jsz, :], in_=o_t[:jsz])

        if R == 1:
            body()
        else:
            with tc.For_i(0, R, 1) as i:
                body(i)

    if drainfix:
        _split_excess_waits(nc, 1)
    return nc


def prep_inputs(x, adj):
    """Host-side shard/layout prep. Returns in_maps for run_bass_kernel_spmd."""
    f8 = ml_dtypes.float8_e4m3
    DT = D // 128
    KT = N_PAD // 128
    x8 = np.zeros((N_PAD, D), dtype=f8)
    x8[:N] = x.astype(f8)
    x8f = x8.astype(np.float32)
    # key inverse norms of the QUANTIZED keys (consistent with the fp8
    # matmul), folded into exp as its per-partition scale
    rn = np.zeros((N_PAD,), dtype=np.float32)
    nrm = np.sqrt((x8f[:N] ** 2).sum(axis=1))
    rn[:N] = 1.0 / np.maximum(nrm, 1e-12)
    rn_s = np.ascontiguousarray(rn.reshape(KT, 128).T)
    # key rows folded to [128, KT, D]: xbf[p, t, d] = x8[t*128+p, d]
    xbf = np.ascontiguousarray(x8.reshape(KT, 128, D).transpose(1, 0, 2))
    # folded transpose [128, DT, N_PAD]: xbT[p, i, n] = x8[n, i*128+p]
    xbT = np.ascontiguousarray(
        x8.T.reshape(DT, 128, N_PAD).transpose(1, 0, 2))
    # normalized queries from the fp32 x, then quantized + folded
    qn = x / np.maximum(np.linalg.norm(x, axis=1, keepdims=True), 1e-12)
    qn8 = np.zeros((N_PAD, D), dtype=f8)
    qn8[:N] = qn.astype(f8)
    in_maps = []
    for c in range(N_CORES):
        q0 = c * NQ
        q1 = min(q0 + NQ, N)
        nreal = max(q1 - q0, 0)
        maskT_c = np.ones((N_PAD, NQ), dtype=np.uint8)
        if nreal > 0:
            maskT_c[:N, :nreal] = (1 - adj[q0:q1, :].T).astype(np.uint8)
            maskT_c[N:, :nreal] = 0
        # fold to [128, KT, NQ]: maskTf[p, t, q] = maskT[t*128+p, q]
        maskTf_c = np.ascontiguousarray(
            maskT_c.reshape(KT, 128, NQ).transpose(1, 0, 2))
        qnT_c = np.ascontiguousarray(
            qn8[q0:q0 + NQ].T.reshape(DT, 128, NQ).transpose(1, 0, 2))
        xq2_c = np.zeros((NQ, D), dtype=np.float32)
        if nreal > 0:
            xq2_c[:nreal] = 2.0 * x[q0:q1]
        in_maps.append({"xb": xbf, "xbT": xbT, "qnT": qnT_c, "rn": rn_s,
                        "maskT": maskTf_c, "xq2": xq2_c})
    return in_maps


_cached = {}


def _get_nc(R=1):
    if R not in _cached:
        _cached[R] = build(R=R)
    return _cached[R]


_neff_cache_installed = False


def _install_neff_cache():
    """Disk-cache walrus NEFF compiles keyed by the BIR JSON hash, so repeat
    processes skip the multi-minute compile."""
    global _neff_cache_installed
    if _neff_cache_installed:
        return
    _neff_cache_installed = True
    import hashlib
    import shutil
    from concourse import bass2jax
    cache_dir = os.path.expanduser("~/.cache/bass_neff_cache")
    os.makedirs(cache_dir, exist_ok=True)
    orig = bass2jax.compile_bir_kernel

    def cached(bir_json, tmpdir, neff_name="file.neff"):
        key = hashlib.sha256(
            bir_json if isinstance(bir_json, bytes) else bir_json.encode()
        ).hexdigest()[:32]
        hit = os.path.join(cache_dir, key + ".neff")
        dst = os.path.join(tmpdir, neff_name)
        if os.path.exists(hit):
            shutil.copyfile(hit, dst)
            return dst
        path = orig(bir_json, tmpdir, neff_name)
        try:
            shutil.copyfile(path, hit)
        except OSError:
            pass
        return path

    bass2jax.compile_bir_kernel = cached


def run_on_cores(in_maps, R=1):
    _install_neff_cache()
    from concourse.bass_utils import run_bass_kernel_spmd
    nc = _get_nc(R)
    res = run_bass_kernel_spmd(nc, in_maps, list(range(N_CORES)))
    return [res.results[c]["out"] for c in range(N_CORES)]


def kernel(x, adj):
    x = np.asarray(x, dtype=np.float32)
    adj = np.asarray(adj, dtype=np.int32)
    assert x.shape == (N, D) and adj.shape == (N, N)
    in_maps = prep_inputs(x, adj)
    outs = run_on_cores(in_maps, R=1)
    full = np.concatenate(outs, axis=0)[:N]
    return np.ascontiguousarray(full.astype(np.float32))
